# revision 5
# baseline (speedup 1.0000x reference)
"""Trainium2 Bass kernel for CurvSelfAttention.

Reference computation (per batch b):
    Q = hs @ Wq + bq ; K = hs @ Wk + bk ; V = hs @ Wv + bv      # [S, H]
    s = sigmoid(hs @ Ws + bs) * 0.2 + 0.9                        # [S, NH*G]
    Q[:, h*64+g*8+r] *= s[:, h*8+g]
    per head h: ctx_h = softmax(Q_h K_h^T / 8) V_h               # [S, 64]
    out = concat_h(ctx_h)                                        # [S, NH*64]

Sharding over 8 cores: core c = (b = c // 2, hh = c % 2); each core owns
batch b and heads hh*8 .. hh*8+8 (512 output columns). No collectives.

Per-core algorithm (HW-measured design rules: per-MM cost = N/2.4 + LDW,
LDW = cols/(4 if K==128 else 1)/1.2 and never hides; EXP = (N+~330)/1.2):

  hsT[k, t]       <- xbar-DMA-transpose of hs (bf16)
  s^T compact     <- Ws^T-stationary matmuls -> sigmoid -> sxc [65, S]
                     (row 64 = ones); expanded x8 + affine (0.2x+0.9) via a
                     selection-matrix matmul (sel [65,128] per jb), folded
                     into the SEL entries - no DRAM bounce.
  K^T [j, t]      <- W^T-stationary matmuls (pair rows: h0 d 0-63, h1 64-127)
  Q^T zero-padded <- same matmuls, epilogue writes (ps+bq)*sexp into
                     q0pad rows 0-63 (rows 64-127 = 0) and q1pad rows 64-127.
                     This lets the scores stationary be the full [128,128]
                     K-pair chunk (FWL fast weight load: 27ns vs 107ns).
  V [t, j]        <- hsT-chunk-stationary matmuls + ones col (denominator)
  scores psum     [128t, 1024]: head0 cols 0-511, head1 512-1023 (2 banks)
  probs           ONE Exp N=1024 per t-chunk covering both heads
  ctx^T [65, 512] per head: [V_h | 1]^T @ probs_h, accumulated over t-chunks
  out             PE-transpose chunks, * 1/denom, DMA per (hp, ssb) window

Loop: hp (4 head pairs) x ssb (4 windows of 512 s) x tc (16 t-chunks).
Projection matmuls for later head pairs drain as fillers inside the loop.
"""

import os
import sys

sys.path.insert(0, "/opt/trn_rl_repo")

import numpy as np
import ml_dtypes
from collections import deque
from contextlib import ExitStack

import concourse.bass as bass
import concourse.bacc as bacc
import concourse.tile as tile
from concourse import mybir
from concourse import bass_utils

F32 = mybir.dt.float32
BF16 = mybir.dt.bfloat16
AF = mybir.ActivationFunctionType
ALU = mybir.AluOpType

P = 128          # SBUF partitions
NB = 512         # matmul moving free-dim block
W2 = 512         # attention s-window
HD = 64          # head dim
G = 8            # groups per head
RING = 6         # probs ring chunks
LAG = 3          # ctx trails exp by this many chunks
SC_MIN, SC_MAX = 0.9, 1.1


def build_bass(S=2048, H=1024, NHL=8):
    """Build the per-core Bass module. NHL = local heads; JL = NHL*64."""
    JL = NHL * HD
    GL = NHL * G           # compact scale channels (64)
    KT = H // P            # contraction k-tiles (8)
    JB = JL // P           # j row-blocks == head pairs (4)
    NTB = S // NB          # 512-wide t blocks (4)
    NTC = S // P           # 128-wide t chunks (16)
    NSSB = S // W2         # s windows per head pair (4)
    HP = NHL // 2          # head pairs (4)

    nc = bacc.Bacc(trn_type="TRN2", target_bir_lowering=False, debug=False,
                   num_devices=8)

    hs = nc.dram_tensor("hs", [S, H], BF16, kind="ExternalInput").ap()
    wq = nc.dram_tensor("wq", [H, JL], BF16, kind="ExternalInput").ap()
    wk = nc.dram_tensor("wk", [H, JL], BF16, kind="ExternalInput").ap()
    wv = nc.dram_tensor("wv", [H, JL], BF16, kind="ExternalInput").ap()
    ws = nc.dram_tensor("ws", [H, GL], BF16, kind="ExternalInput").ap()
    bq = nc.dram_tensor("bq", [JL], F32, kind="ExternalInput").ap()
    bk = nc.dram_tensor("bk", [JL], F32, kind="ExternalInput").ap()
    bv = nc.dram_tensor("bv", [JL], F32, kind="ExternalInput").ap()
    bs = nc.dram_tensor("bs", [GL], F32, kind="ExternalInput").ap()
    ident = nc.dram_tensor("ident", [P, P], F32, kind="ExternalInput").ap()
    sel = nc.dram_tensor("sel", [GL + 1, JL], BF16, kind="ExternalInput").ap()
    out = nc.dram_tensor("out", [S, JL], F32, kind="ExternalOutput").ap()

    with tile.TileContext(nc) as tc, ExitStack() as ctx:
        cpool = ctx.enter_context(tc.tile_pool(name="consts", bufs=1))
        qkpool = ctx.enter_context(tc.tile_pool(name="qk", bufs=1))
        vpool = ctx.enter_context(tc.tile_pool(name="v", bufs=1))
        sxpool = ctx.enter_context(tc.tile_pool(name="sexp", bufs=1))
        hpool = ctx.enter_context(tc.tile_pool(name="hsT", bufs=1))
        wpool = ctx.enter_context(tc.tile_pool(name="wts", bufs=1))
        ppsum = ctx.enter_context(tc.tile_pool(name="ppsum", bufs=2,
                                               space="PSUM"))
        ptmp = ctx.enter_context(tc.tile_pool(name="ptmp", bufs=2))

        # persistent activation tensors
        # q pads: q0pad[hp] rows 0-63 live (head 2hp), rows 64-127 zero;
        #         q1pad[hp] rows 64-127 live (head 2hp+1), rows 0-63 zero
        q0pad = [qkpool.tile([P, S], BF16, tag=f"q0p{hp}", name=f"q0p{hp}")
                 for hp in range(HP)]
        q1pad = [qkpool.tile([P, S], BF16, tag=f"q1p{hp}", name=f"q1p{hp}")
                 for hp in range(HP)]
        k_sb = [qkpool.tile([P, S], BF16, tag=f"k{jb}", name=f"k{jb}")
                for jb in range(JB)]
        # V as [t-chunk][128, NHL, 65]; col 64 of each head = ones (denom)
        v_sb = [vpool.tile([P, NHL, HD + 1], BF16, tag=f"v{i}", name=f"v{i}")
                for i in range(NTC)]
        sexp = [sxpool.tile([P, S], BF16, tag=f"sx{jb}", name=f"sx{jb}")
                for jb in range(JB)]
        sxc = ptmp.tile([GL + 1, S], BF16, tag="sxc", bufs=1)

        # big one-time memsets go on the (otherwise idle) gpsimd engine
        for hp in range(HP):
            nc.gpsimd.memset(q0pad[hp][HD:P, :], 0.0)
            nc.gpsimd.memset(q1pad[hp][0:HD, :], 0.0)
        nc.gpsimd.memset(sxc[GL:GL + 1, :], 1.0)
        for i in range(NTC):
            nc.gpsimd.memset(v_sb[i][:, :, HD], 1.0)

        # ---- input loads. DMA_TRANSPOSE mutually excludes all other DMA
        # traffic (deadlock workaround), so the 8 hsT transposes are the
        # FIRST DMAs emitted anywhere, back-to-back on the sync queue with
        # nothing else in flight. All other loads are emitted after them:
        # weights stream on the sync queue, consts/sel/wv in parallel on
        # the vector engine's queue. ----
        hsT = []
        for k in range(KT):
            t = hpool.tile([P, S], BF16, tag=f"hsT{k}", name=f"hsT{k}")
            nc.sync.dma_start_transpose(t[:], hs[:, k * P:(k + 1) * P])
            hsT.append(t)
        ws_sb = []
        for k in range(KT):
            t = wpool.tile([P, GL], BF16, tag=f"ws{k}", name=f"ws{k}")
            nc.sync.dma_start(t[:], ws[k * P:(k + 1) * P, :])
            ws_sb.append(t)
        # consts on the vector queue (runs in parallel with sync weights)
        bq_sb = []
        bk_sb = []
        for jb in range(JB):
            t = cpool.tile([P, 1], F32, tag=f"bq{jb}")
            nc.scalar.dma_start(
                t[:], bq[jb * P:(jb + 1) * P].rearrange("(a b) -> a b", b=1))
            bq_sb.append(t)
            t = cpool.tile([P, 1], F32, tag=f"bk{jb}")
            nc.scalar.dma_start(
                t[:], bk[jb * P:(jb + 1) * P].rearrange("(a b) -> a b", b=1))
            bk_sb.append(t)
        bs_sb = cpool.tile([GL, 1], F32, tag="bs")
        nc.scalar.dma_start(bs_sb[:], bs.rearrange("(a b) -> a b", b=1))
        idf = cpool.tile([P, P], F32, tag="idf")
        nc.scalar.dma_start(idf[:], ident)
        bvb = cpool.tile([P, JL], F32, tag="bvb")
        nc.scalar.dma_start(
            bvb[:], bv.rearrange("(a b) -> a b", a=1).broadcast_to([P, JL]))
        sel_sb = cpool.tile([GL + 1, JL], BF16, tag="sel")
        nc.scalar.dma_start(sel_sb[:], sel)
        wq_sb, wk_sb, wv_sb = [], [], []
        for k in range(KT):
            for lst in (wk_sb, wq_sb, wv_sb):
                lst.append(None)
        for k in range(KT):
            for name, dram, lst, q in (("wk", wk, wk_sb, nc.sync),
                                       ("wq", wq, wq_sb, nc.sync)):
                t = wpool.tile([P, JL], BF16, tag=f"{name}{k}",
                               name=f"{name}{k}")
                q.dma_start(t[:], dram[k * P:(k + 1) * P, :])
                lst[k] = t
        for k in range(KT):
            t = wpool.tile([P, JL], BF16, tag=f"wv{k}", name=f"wv{k}")
            nc.scalar.dma_start(t[:], wv[k * P:(k + 1) * P, :])
            wv_sb[k] = t

        # ---- projection work units (4 matmuls each), emitted either in
        # the prefix or interleaved into the attention loop ----
        open_ps = {}

        def emit_qk_unit(jb, kind, tb, half):
            wlist = wq_sb if kind == "q" else wk_sb
            key = (jb, kind, tb)
            if half == 0:
                open_ps[key] = ppsum.tile([P, NB], F32, tag="pp", name="pp")
            ps = open_ps[key]
            kh = KT // 2
            for k in range(kh * half, kh * half + kh):
                nc.tensor.matmul(
                    ps[:], wlist[k][:, jb * P:(jb + 1) * P],
                    hsT[k][:, tb * NB:(tb + 1) * NB],
                    start=(k == 0), stop=(k == KT - 1))
            if half == 1:
                del open_ps[key]
                if kind == "q":
                    nc.vector.scalar_tensor_tensor(
                        q0pad[jb][0:HD, tb * NB:(tb + 1) * NB], ps[0:HD, :],
                        bq_sb[jb][0:HD], sexp[jb][0:HD, tb * NB:(tb + 1) * NB],
                        ALU.add, ALU.mult)
                    nc.vector.scalar_tensor_tensor(
                        q1pad[jb][HD:P, tb * NB:(tb + 1) * NB], ps[HD:P, :],
                        bq_sb[jb][HD:P], sexp[jb][HD:P, tb * NB:(tb + 1) * NB],
                        ALU.add, ALU.mult)
                else:
                    nc.vector.tensor_scalar_add(
                        k_sb[jb][:, tb * NB:(tb + 1) * NB], ps[:], bk_sb[jb][:])

        def emit_v_unit(tc_, half):
            key = ("v", tc_)
            if half == 0:
                open_ps[key] = ppsum.tile([P, JL], F32, tag="pp", name="pp")
            ps = open_ps[key]
            kh = KT // 2
            for k in range(kh * half, kh * half + kh):
                nc.tensor.matmul(
                    ps[:], hsT[k][:, tc_ * P:(tc_ + 1) * P], wv_sb[k][:],
                    start=(k == 0), stop=(k == KT - 1))
            if half == 1:
                del open_ps[key]
                for h in range(NHL):
                    nc.vector.tensor_add(
                        v_sb[tc_][:, h, 0:HD], ps[:, h * HD:(h + 1) * HD],
                        bvb[:, h * HD:(h + 1) * HD])

        # ---- prefix: compact scale -> sigmoid -> SEL-expand, then K jb0
        # (all tb) and Q jb0 (tb 0,1) ----
        for tb in range(NTB):
            ps = ppsum.tile([P, NB], F32, tag="pp", name="pp")
            for k in range(KT):
                nc.tensor.matmul(
                    ps[0:GL, :], ws_sb[k][:],
                    hsT[k][:, tb * NB:(tb + 1) * NB],
                    start=(k == 0), stop=(k == KT - 1))
            nc.scalar.activation(sxc[0:GL, tb * NB:(tb + 1) * NB],
                                 ps[0:GL, :], AF.Sigmoid, bias=bs_sb[:])
        # expand x8 + affine via SEL matmul: sexp[jb] = SEL_jb^T @ sxc
        for jb in range(JB):
            for tb in range(NTB):
                ps = ppsum.tile([P, NB], F32, tag="pp", name="pp")
                nc.tensor.matmul(
                    ps[:], sel_sb[:, jb * P:(jb + 1) * P],
                    sxc[:, tb * NB:(tb + 1) * NB], start=True, stop=True)
                nc.vector.tensor_copy(sexp[jb][:, tb * NB:(tb + 1) * NB],
                                      ps[:])
        for tb in range(NTB):
            for half in range(2):
                emit_qk_unit(0, "k", tb, half)
        for tb in range(2):
            for half in range(2):
                emit_qk_unit(0, "q", tb, half)

        # deferred projection units, drained as attention filler
        projq = deque()
        for tc_ in range(NTC):
            for half in range(2):
                projq.append(("v", tc_, half))
        for tb in range(2, NTB):
            for half in range(2):
                projq.append(("qk", 0, "q", tb, half))
        for jb in range(1, JB):
            for kind in ("k", "q"):
                for tb in range(NTB):
                    for half in range(2):
                        projq.append(("qk", jb, kind, tb, half))

        def drain_proj(n):
            for _ in range(n):
                if not projq:
                    return
                u = projq.popleft()
                if u[0] == "v":
                    emit_v_unit(u[1], u[2])
                else:
                    emit_qk_unit(u[1], u[2], u[3], u[4])

        # ================= attention =================
        # PSUM: scores ring 2x2 banks + ctx 2x1 + pp 2x1 = 8 banks
        with tc.tile_pool(name="probs", bufs=1) as prpool, \
             tc.tile_pool(name="asm", bufs=2) as apool, \
             tc.tile_pool(name="spsum", bufs=2, space="PSUM") as spsum, \
             tc.tile_pool(name="cpsum", bufs=1, space="PSUM") as cpsum, \
             tc.tile_pool(name="ctmp", bufs=2) as ctpool, \
             tc.tile_pool(name="rtmp", bufs=2) as rtpool:

            # per-(hp, ssb) filler quota (units of 4 matmuls per chunk iter)
            quota = {}
            for hp in range(HP):
                for ssb in range(NSSB):
                    quota[(hp, ssb)] = 0
            quota[(0, 0)] = 2
            quota[(0, 1)] = 1
            quota[(0, 2)] = 1
            quota[(0, 3)] = 1
            quota[(1, 0)] = 1
            quota[(1, 1)] = 1

            # deferred per-window tail work (transpose + normalize + DMA),
            # drained one piece per iteration of the NEXT window
            tailq = deque()

            def drain_tail(n):
                for _ in range(n):
                    if not tailq:
                        return
                    tailq.popleft()()

            def make_tail(hp, ssb, cc, cts):
                def run():
                    asm = apool.tile([P, P], F32, tag=f"asm{cc}",
                                     name=f"asm{cc}")
                    for i in range(2):
                        trp = ppsum.tile([P, NB], F32, tag="pp", name="trp")
                        trv = trp[:, 0:HD + 1]
                        nc.tensor.transpose(
                            trv, cts[i][:, cc * P:(cc + 1) * P],
                            idf[0:HD + 1, 0:HD + 1])
                        rc = rtpool.tile([P, 1], F32, tag="rc")
                        nc.vector.reciprocal(rc[:], trv[:, HD:HD + 1])
                        nc.vector.tensor_scalar_mul(
                            asm[:, i * HD:(i + 1) * HD], trv[:, 0:HD], rc[:])
                    srow = ssb * W2 + cc * P
                    nc.sync.dma_start(
                        out[srow:srow + P, hp * P:(hp + 1) * P], asm[:])
                return run

            for hp in range(HP):
                qpads = (q0pad[hp], q1pad[hp])
                for ssb in range(NSSB):
                    scol = ssb * W2
                    nq = quota[(hp, ssb)]
                    pts = prpool.tile([P, RING, 2 * W2], BF16, tag="pts",
                                      name="pts")
                    cps = [cpsum.tile([P, W2], F32, tag=f"c{i}", name=f"c{i}")
                           for i in range(2)]

                    def ctx_chunk(tcc):
                        for i in range(2):
                            h = hp * 2 + i
                            nc.tensor.matmul(
                                cps[i][0:HD + 1, :],
                                v_sb[tcc][:, h, :],
                                pts[:, tcc % RING, i * W2:(i + 1) * W2],
                                start=(tcc == 0), stop=(tcc == NTC - 1))

                    for tc_ in range(NTC):
                        if tc_ >= LAG:
                            ctx_chunk(tc_ - LAG)
                        drain_tail(1)
                        drain_proj(nq)
                        sc = spsum.tile([P, 2 * W2], F32, tag="sc", name="sc")
                        for i in range(2):
                            nc.tensor.matmul(
                                sc[:, i * W2:(i + 1) * W2],
                                k_sb[hp][:, tc_ * P:(tc_ + 1) * P],
                                qpads[i][:, scol:scol + W2],
                                start=True, stop=True)
                        nc.scalar.activation(
                            pts[:, tc_ % RING, :], sc[:], AF.Exp,
                            scale=1.0 / 8.0)
                    for tcc in range(NTC - LAG, NTC):
                        ctx_chunk(tcc)

                    # evacuate ctx psum to SBUF now; defer the transposes,
                    # normalization and output DMA into the next window
                    cts = []
                    for i in range(2):
                        ct = ctpool.tile([HD + 1, W2], F32, tag=f"ct{i}")
                        nc.vector.tensor_copy(ct[:], cps[i][0:HD + 1, :])
                        cts.append(ct)
                    for cc in range(W2 // P):
                        tailq.append(make_tail(hp, ssb, cc, cts))
            while tailq:
                tailq.popleft()()

    nc.finalize()
    return nc


_CACHE = {}


def _get_nc():
    if "nc" not in _CACHE:
        _CACHE["nc"] = build_bass()
    return _CACHE["nc"]


def _make_sel():
    """SEL [65, 512]: sexp[jb][j, t] = 0.2*sig[c(jb,j), t] + 0.9 where
    c(jb, j) = (2*jb + j//64)*8 + (j%64)//8."""
    sel = np.zeros((65, 512), dtype=np.float32)
    for jb in range(4):
        for j in range(128):
            c = (2 * jb + j // 64) * 8 + (j % 64) // 8
            sel[c, jb * 128 + j] = SC_MAX - SC_MIN
    sel[64, :] = SC_MIN
    return sel


def _shard(inputs):
    """Split full inputs into 8 per-core input maps (host-side, bf16 cast)."""
    hidden_states = inputs["hidden_states"]
    Wq, bq = inputs["Wq"], inputs["bq"]
    Wk, bk = inputs["Wk"], inputs["bk"]
    Wv, bv = inputs["Wv"], inputs["bv"]
    Ws, bs = inputs["Ws"], inputs["bs"]
    JL = 512   # output cols per core
    GL = 64    # Ws cols per core
    bf = ml_dtypes.bfloat16
    ident = np.eye(P, dtype=np.float32)
    sel = _make_sel().astype(bf)
    in_maps = []
    for c in range(8):
        b, hh = c // 2, c % 2
        in_maps.append({
            "hs": np.ascontiguousarray(hidden_states[b]).astype(bf),
            "wq": np.ascontiguousarray(Wq[:, hh * JL:(hh + 1) * JL]).astype(bf),
            "wk": np.ascontiguousarray(Wk[:, hh * JL:(hh + 1) * JL]).astype(bf),
            "wv": np.ascontiguousarray(Wv[:, hh * JL:(hh + 1) * JL]).astype(bf),
            "ws": np.ascontiguousarray(Ws[:, hh * GL:(hh + 1) * GL]).astype(bf),
            "bq": np.ascontiguousarray(bq[hh * JL:(hh + 1) * JL]).astype(np.float32),
            "bk": np.ascontiguousarray(bk[hh * JL:(hh + 1) * JL]).astype(np.float32),
            "bv": np.ascontiguousarray(bv[hh * JL:(hh + 1) * JL]).astype(np.float32),
            "bs": np.ascontiguousarray(bs[hh * GL:(hh + 1) * GL]).astype(np.float32),
            "ident": ident,
            "sel": sel,
        })
    return in_maps


def kernel(hidden_states, Wq, bq, Wk, bk, Wv, bv, Ws, bs):
    B, S, H = hidden_states.shape
    NH = 16
    JL = 512
    nc = _get_nc()
    in_maps = _shard(dict(hidden_states=hidden_states, Wq=Wq, bq=bq, Wk=Wk,
                          bk=bk, Wv=Wv, bv=bv, Ws=Ws, bs=bs))

    res = bass_utils.run_bass_kernel_spmd(nc, in_maps, core_ids=list(range(8)))

    outp = np.zeros((B, S, NH * HD), dtype=np.float32)
    for c in range(8):
        b, hh = c // 2, c % 2
        outp[b][:, hh * JL:(hh + 1) * JL] = res.results[c]["out"]
    return outp



# revision 8
# speedup vs baseline: 1.2930x; 1.2930x over previous
"""Trainium2 Bass kernel for CurvSelfAttention.

Reference computation (per batch b):
    Q = hs @ Wq + bq ; K = hs @ Wk + bk ; V = hs @ Wv + bv      # [S, H]
    s = sigmoid(hs @ Ws + bs) * 0.2 + 0.9                        # [S, NH*G]
    Q[:, h*64+g*8+r] *= s[:, h*8+g]
    per head h: ctx_h = softmax(Q_h K_h^T / 8) V_h               # [S, 64]
    out = concat_h(ctx_h)                                        # [S, NH*64]

Sharding over 8 cores: core c = (b = c // 2, hh = c % 2); each core owns
batch b and heads hh*8 .. hh*8+8 (512 output columns). No collectives.

Per-core algorithm (HW-measured design rules: per-MM cost = N/2.4 + LDW,
LDW = cols/(4 if K==128 else 1)/1.2 and never hides; EXP = (N+~330)/1.2):

  hsT[k, t]       <- xbar-DMA-transpose of hs (bf16)
  s^T compact     <- Ws^T-stationary matmuls -> sigmoid -> sxc [65, S]
                     (row 64 = ones); expanded x8 + affine (0.2x+0.9) via a
                     selection-matrix matmul (sel [65,128] per jb), folded
                     into the SEL entries - no DRAM bounce.
  K^T [j, t]      <- W^T-stationary matmuls (pair rows: h0 d 0-63, h1 64-127)
  Q^T zero-padded <- same matmuls, epilogue writes (ps+bq)*sexp into
                     q0pad rows 0-63 (rows 64-127 = 0) and q1pad rows 64-127.
                     This lets the scores stationary be the full [128,128]
                     K-pair chunk (FWL fast weight load: 27ns vs 107ns).
  V [t, j]        <- hsT-chunk-stationary matmuls + ones col (denominator)
  scores psum     [128t, 1024]: head0 cols 0-511, head1 512-1023 (2 banks)
  probs           ONE Exp N=1024 per t-chunk covering both heads
  ctx^T [65, 512] per head: [V_h | 1]^T @ probs_h, accumulated over t-chunks
  out             PE-transpose chunks, * 1/denom, DMA per (hp, ssb) window

Loop: hp (4 head pairs) x ssb (4 windows of 512 s) x tc (16 t-chunks).
Projection matmuls for later head pairs drain as fillers inside the loop.
"""

import os
import sys

sys.path.insert(0, "/opt/trn_rl_repo")

import numpy as np
import ml_dtypes
from collections import deque
from contextlib import ExitStack

import concourse.bass as bass
import concourse.bacc as bacc
import concourse.tile as tile
from concourse import mybir
from concourse import bass_utils

F32 = mybir.dt.float32
BF16 = mybir.dt.bfloat16
AF = mybir.ActivationFunctionType
ALU = mybir.AluOpType

P = 128          # SBUF partitions
NB = 512         # matmul moving free-dim block
W2 = 512         # attention s-window
HD = 64          # head dim
G = 8            # groups per head
RING = 6         # probs ring chunks
LAG = 3          # ctx trails exp by this many chunks
SC_MIN, SC_MAX = 0.9, 1.1


def build_bass(S=2048, H=1024, NHL=8):
    """Build the per-core Bass module. NHL = local heads; JL = NHL*64."""
    JL = NHL * HD
    GL = NHL * G           # compact scale channels (64)
    KT = H // P            # contraction k-tiles (8)
    JB = JL // P           # j row-blocks == head pairs (4)
    NTB = S // NB          # 512-wide t blocks (4)
    NTC = S // P           # 128-wide t chunks (16)
    NSSB = S // W2         # s windows per head pair (4)
    HP = NHL // 2          # head pairs (4)

    nc = bacc.Bacc(trn_type="TRN2", target_bir_lowering=False, debug=False,
                   num_devices=8)

    hs = nc.dram_tensor("hs", [H, S], BF16, kind="ExternalInput").ap()
    wq = nc.dram_tensor("wq", [H, JL], BF16, kind="ExternalInput").ap()
    wk = nc.dram_tensor("wk", [H, JL], BF16, kind="ExternalInput").ap()
    wv = nc.dram_tensor("wv", [H, JL], BF16, kind="ExternalInput").ap()
    ws = nc.dram_tensor("ws", [H, GL], BF16, kind="ExternalInput").ap()
    bq = nc.dram_tensor("bq", [JL], F32, kind="ExternalInput").ap()
    bk = nc.dram_tensor("bk", [JL], F32, kind="ExternalInput").ap()
    bv = nc.dram_tensor("bv", [JL], F32, kind="ExternalInput").ap()
    bs = nc.dram_tensor("bs", [GL], F32, kind="ExternalInput").ap()
    ident = nc.dram_tensor("ident", [P, P], F32, kind="ExternalInput").ap()
    sel = nc.dram_tensor("sel", [GL + 1, JL], BF16, kind="ExternalInput").ap()
    out = nc.dram_tensor("out", [S, JL], F32, kind="ExternalOutput").ap()

    with tile.TileContext(nc) as tc, ExitStack() as ctx:
        cpool = ctx.enter_context(tc.tile_pool(name="consts", bufs=1))
        qkpool = ctx.enter_context(tc.tile_pool(name="qk", bufs=1))
        vpool = ctx.enter_context(tc.tile_pool(name="v", bufs=1))
        sxpool = ctx.enter_context(tc.tile_pool(name="sexp", bufs=1))
        hpool = ctx.enter_context(tc.tile_pool(name="hsT", bufs=1))
        wpool = ctx.enter_context(tc.tile_pool(name="wts", bufs=1))
        ppsum = ctx.enter_context(tc.tile_pool(name="ppsum", bufs=2,
                                               space="PSUM"))
        ptmp = ctx.enter_context(tc.tile_pool(name="ptmp", bufs=2))

        # persistent activation tensors
        # q pads: q0pad[hp] rows 0-63 live (head 2hp), rows 64-127 zero;
        #         q1pad[hp] rows 64-127 live (head 2hp+1), rows 0-63 zero
        q0pad = [qkpool.tile([P, S], BF16, tag=f"q0p{hp}", name=f"q0p{hp}")
                 for hp in range(HP)]
        q1pad = [qkpool.tile([P, S], BF16, tag=f"q1p{hp}", name=f"q1p{hp}")
                 for hp in range(HP)]
        k_sb = [qkpool.tile([P, S], BF16, tag=f"k{jb}", name=f"k{jb}")
                for jb in range(JB)]
        # V as [t-chunk][128, NHL, 65]; col 64 of each head = ones (denom)
        v_sb = [vpool.tile([P, NHL, HD + 1], BF16, tag=f"v{i}", name=f"v{i}")
                for i in range(NTC)]
        sexp = [sxpool.tile([P, S], BF16, tag=f"sx{jb}", name=f"sx{jb}")
                for jb in range(JB)]
        sxc = ptmp.tile([GL + 1, S], BF16, tag="sxc", bufs=1)

        # ---- input loads. hs arrives pre-transposed from the host
        # ([H, S]), so there are NO DMA transposes and the three DMA-capable
        # queues (sync, scalar/ACT, gpsimd) stream in parallel:
        #   sync:   hsT chunks (4MB, the bulk)
        #   scalar: ws, bs, wk, bk, sel (scale/sigmoid + K path)
        #   gpsimd: early memsets, wq, bq, wv, bvb, idf, late memsets
        hsT = []
        for k in range(KT):
            t = hpool.tile([P, S], BF16, tag=f"hsT{k}", name=f"hsT{k}")
            nc.sync.dma_start(t[:], hs[k * P:(k + 1) * P, :])
            hsT.append(t)
        ws_sb = []
        for k in range(KT):
            t = wpool.tile([P, GL], BF16, tag=f"ws{k}", name=f"ws{k}")
            nc.scalar.dma_start(t[:], ws[k * P:(k + 1) * P, :])
            ws_sb.append(t)
        bs_sb = cpool.tile([GL, 1], F32, tag="bs")
        nc.scalar.dma_start(bs_sb[:], bs.rearrange("(a b) -> a b", b=1))
        wq_sb, wk_sb, wv_sb = [None] * KT, [None] * KT, [None] * KT
        bk_sb = []
        for k in range(KT):
            t = wpool.tile([P, JL], BF16, tag=f"wk{k}", name=f"wk{k}")
            nc.scalar.dma_start(t[:], wk[k * P:(k + 1) * P, :])
            wk_sb[k] = t
        for jb in range(JB):
            t = cpool.tile([P, 1], F32, tag=f"bk{jb}")
            nc.scalar.dma_start(
                t[:], bk[jb * P:(jb + 1) * P].rearrange("(a b) -> a b", b=1))
            bk_sb.append(t)
        sel_sb = cpool.tile([GL + 1, JL], BF16, tag="sel")
        nc.scalar.dma_start(sel_sb[:], sel)

        # gpsimd queue: memsets needed first, then Q/V-side loads, then the
        # remaining one-time memsets
        nc.gpsimd.memset(sxc[GL:GL + 1, :], 1.0)
        nc.gpsimd.memset(q0pad[0][HD:P, :], 0.0)
        nc.gpsimd.memset(q1pad[0][0:HD, :], 0.0)
        bq_sb = []
        for k in range(KT):
            t = wpool.tile([P, JL], BF16, tag=f"wq{k}", name=f"wq{k}")
            nc.gpsimd.dma_start(t[:], wq[k * P:(k + 1) * P, :])
            wq_sb[k] = t
        for jb in range(JB):
            t = cpool.tile([P, 1], F32, tag=f"bq{jb}")
            nc.gpsimd.dma_start(
                t[:], bq[jb * P:(jb + 1) * P].rearrange("(a b) -> a b", b=1))
            bq_sb.append(t)
        for k in range(KT):
            t = wpool.tile([P, JL], BF16, tag=f"wv{k}", name=f"wv{k}")
            nc.gpsimd.dma_start(t[:], wv[k * P:(k + 1) * P, :])
            wv_sb[k] = t
        bvb = cpool.tile([P, JL], F32, tag="bvb")
        nc.gpsimd.dma_start(
            bvb[:], bv.rearrange("(a b) -> a b", a=1).broadcast_to([P, JL]))
        idf = cpool.tile([P, P], F32, tag="idf")
        nc.gpsimd.dma_start(idf[:], ident)
        for hp in range(1, HP):
            nc.gpsimd.memset(q0pad[hp][HD:P, :], 0.0)
            nc.gpsimd.memset(q1pad[hp][0:HD, :], 0.0)
        for i in range(NTC):
            nc.gpsimd.memset(v_sb[i][:, :, HD], 1.0)

        # ---- projection work units (4 matmuls each), emitted either in
        # the prefix or interleaved into the attention loop ----
        open_ps = {}

        def emit_qk_unit(jb, kind, tb, half):
            wlist = wq_sb if kind == "q" else wk_sb
            key = (jb, kind, tb)
            if half == 0:
                open_ps[key] = ppsum.tile([P, NB], F32, tag="pp", name="pp")
            ps = open_ps[key]
            kh = KT // 2
            for k in range(kh * half, kh * half + kh):
                nc.tensor.matmul(
                    ps[:], wlist[k][:, jb * P:(jb + 1) * P],
                    hsT[k][:, tb * NB:(tb + 1) * NB],
                    start=(k == 0), stop=(k == KT - 1))
            if half == 1:
                del open_ps[key]
                if kind == "q":
                    nc.vector.scalar_tensor_tensor(
                        q0pad[jb][0:HD, tb * NB:(tb + 1) * NB], ps[0:HD, :],
                        bq_sb[jb][0:HD], sexp[jb][0:HD, tb * NB:(tb + 1) * NB],
                        ALU.add, ALU.mult)
                    nc.vector.scalar_tensor_tensor(
                        q1pad[jb][HD:P, tb * NB:(tb + 1) * NB], ps[HD:P, :],
                        bq_sb[jb][HD:P], sexp[jb][HD:P, tb * NB:(tb + 1) * NB],
                        ALU.add, ALU.mult)
                else:
                    nc.vector.tensor_scalar_add(
                        k_sb[jb][:, tb * NB:(tb + 1) * NB], ps[:], bk_sb[jb][:])

        def emit_v_unit(tc_, half):
            key = ("v", tc_)
            if half == 0:
                open_ps[key] = ppsum.tile([P, JL], F32, tag="pp", name="pp")
            ps = open_ps[key]
            kh = KT // 2
            for k in range(kh * half, kh * half + kh):
                nc.tensor.matmul(
                    ps[:], hsT[k][:, tc_ * P:(tc_ + 1) * P], wv_sb[k][:],
                    start=(k == 0), stop=(k == KT - 1))
            if half == 1:
                del open_ps[key]
                for h in range(NHL):
                    nc.vector.tensor_add(
                        v_sb[tc_][:, h, 0:HD], ps[:, h * HD:(h + 1) * HD],
                        bvb[:, h * HD:(h + 1) * HD])

        # ---- prefix: compact scale -> sigmoid -> SEL-expand, then K jb0
        # (all tb) and Q jb0 (tb 0,1) ----
        for tb in range(NTB):
            ps = ppsum.tile([P, NB], F32, tag="pp", name="pp")
            for k in range(KT):
                nc.tensor.matmul(
                    ps[0:GL, :], ws_sb[k][:],
                    hsT[k][:, tb * NB:(tb + 1) * NB],
                    start=(k == 0), stop=(k == KT - 1))
            nc.scalar.activation(sxc[0:GL, tb * NB:(tb + 1) * NB],
                                 ps[0:GL, :], AF.Sigmoid, bias=bs_sb[:])
        # expand x8 + affine via SEL matmul: sexp[jb] = SEL_jb^T @ sxc
        for jb in range(JB):
            for tb in range(NTB):
                ps = ppsum.tile([P, NB], F32, tag="pp", name="pp")
                nc.tensor.matmul(
                    ps[:], sel_sb[:, jb * P:(jb + 1) * P],
                    sxc[:, tb * NB:(tb + 1) * NB], start=True, stop=True)
                nc.vector.tensor_copy(sexp[jb][:, tb * NB:(tb + 1) * NB],
                                      ps[:])
        for tb in range(NTB):
            for half in range(2):
                emit_qk_unit(0, "k", tb, half)
        for tb in range(2):
            for half in range(2):
                emit_qk_unit(0, "q", tb, half)

        # deferred projection units, drained as attention filler
        projq = deque()
        for tc_ in range(NTC):
            for half in range(2):
                projq.append(("v", tc_, half))
        for tb in range(2, NTB):
            for half in range(2):
                projq.append(("qk", 0, "q", tb, half))
        for jb in range(1, JB):
            for kind in ("k", "q"):
                for tb in range(NTB):
                    for half in range(2):
                        projq.append(("qk", jb, kind, tb, half))

        def drain_proj(n):
            for _ in range(n):
                if not projq:
                    return
                u = projq.popleft()
                if u[0] == "v":
                    emit_v_unit(u[1], u[2])
                else:
                    emit_qk_unit(u[1], u[2], u[3], u[4])

        # ================= attention =================
        # PSUM: scores ring 2x2 banks + ctx 2x1 + pp 2x1 = 8 banks
        with tc.tile_pool(name="probs", bufs=1) as prpool, \
             tc.tile_pool(name="asm", bufs=2) as apool, \
             tc.tile_pool(name="spsum", bufs=2, space="PSUM") as spsum, \
             tc.tile_pool(name="cpsum", bufs=1, space="PSUM") as cpsum, \
             tc.tile_pool(name="ctmp", bufs=2) as ctpool, \
             tc.tile_pool(name="rtmp", bufs=2) as rtpool:

            # per-(hp, ssb) filler quota (units of 4 matmuls per chunk iter)
            quota = {}
            for hp in range(HP):
                for ssb in range(NSSB):
                    quota[(hp, ssb)] = 0
            quota[(0, 0)] = 2
            quota[(0, 1)] = 1
            quota[(0, 2)] = 1
            quota[(0, 3)] = 1
            quota[(1, 0)] = 1
            quota[(1, 1)] = 1

            # deferred per-window tail work (transpose + normalize + DMA),
            # drained one piece per iteration of the NEXT window
            tailq = deque()

            def drain_tail(n):
                for _ in range(n):
                    if not tailq:
                        return
                    tailq.popleft()()

            def make_tail(hp, ssb, cc, cts):
                def run():
                    asm = apool.tile([P, P], F32, tag=f"asm{cc}",
                                     name=f"asm{cc}")
                    for i in range(2):
                        trp = ppsum.tile([P, NB], F32, tag="pp", name="trp")
                        trv = trp[:, 0:HD + 1]
                        nc.tensor.transpose(
                            trv, cts[i][:, cc * P:(cc + 1) * P],
                            idf[0:HD + 1, 0:HD + 1])
                        rc = rtpool.tile([P, 1], F32, tag="rc")
                        nc.vector.reciprocal(rc[:], trv[:, HD:HD + 1])
                        nc.vector.tensor_scalar_mul(
                            asm[:, i * HD:(i + 1) * HD], trv[:, 0:HD], rc[:])
                    srow = ssb * W2 + cc * P
                    nc.sync.dma_start(
                        out[srow:srow + P, hp * P:(hp + 1) * P], asm[:])
                return run

            for hp in range(HP):
                qpads = (q0pad[hp], q1pad[hp])
                for ssb in range(NSSB):
                    scol = ssb * W2
                    nq = quota[(hp, ssb)]
                    pts = prpool.tile([P, RING, 2 * W2], BF16, tag="pts",
                                      name="pts")
                    cps = [cpsum.tile([P, W2], F32, tag=f"c{i}", name=f"c{i}")
                           for i in range(2)]

                    def ctx_chunk(tcc):
                        for i in range(2):
                            h = hp * 2 + i
                            nc.tensor.matmul(
                                cps[i][0:HD + 1, :],
                                v_sb[tcc][:, h, :],
                                pts[:, tcc % RING, i * W2:(i + 1) * W2],
                                start=(tcc == 0), stop=(tcc == NTC - 1))

                    for tc_ in range(NTC):
                        if tc_ >= LAG:
                            ctx_chunk(tc_ - LAG)
                        drain_tail(1)
                        drain_proj(nq)
                        sc = spsum.tile([P, 2 * W2], F32, tag="sc", name="sc")
                        for i in range(2):
                            nc.tensor.matmul(
                                sc[:, i * W2:(i + 1) * W2],
                                k_sb[hp][:, tc_ * P:(tc_ + 1) * P],
                                qpads[i][:, scol:scol + W2],
                                start=True, stop=True)
                        nc.scalar.activation(
                            pts[:, tc_ % RING, :], sc[:], AF.Exp,
                            scale=1.0 / 8.0)
                    for tcc in range(NTC - LAG, NTC):
                        ctx_chunk(tcc)

                    # evacuate ctx psum to SBUF now; defer the transposes,
                    # normalization and output DMA into the next window
                    cts = []
                    for i in range(2):
                        ct = ctpool.tile([HD + 1, W2], F32, tag=f"ct{i}")
                        nc.vector.tensor_copy(ct[:], cps[i][0:HD + 1, :])
                        cts.append(ct)
                    for cc in range(W2 // P):
                        tailq.append(make_tail(hp, ssb, cc, cts))
            while tailq:
                tailq.popleft()()

    nc.finalize()
    return nc


_CACHE = {}


def _get_nc():
    if "nc" not in _CACHE:
        _CACHE["nc"] = build_bass()
    return _CACHE["nc"]


def _make_sel():
    """SEL [65, 512]: sexp[jb][j, t] = 0.2*sig[c(jb,j), t] + 0.9 where
    c(jb, j) = (2*jb + j//64)*8 + (j%64)//8."""
    sel = np.zeros((65, 512), dtype=np.float32)
    for jb in range(4):
        for j in range(128):
            c = (2 * jb + j // 64) * 8 + (j % 64) // 8
            sel[c, jb * 128 + j] = SC_MAX - SC_MIN
    sel[64, :] = SC_MIN
    return sel


def _shard(inputs):
    """Split full inputs into 8 per-core input maps (host-side, bf16 cast)."""
    hidden_states = inputs["hidden_states"]
    Wq, bq = inputs["Wq"], inputs["bq"]
    Wk, bk = inputs["Wk"], inputs["bk"]
    Wv, bv = inputs["Wv"], inputs["bv"]
    Ws, bs = inputs["Ws"], inputs["bs"]
    JL = 512   # output cols per core
    GL = 64    # Ws cols per core
    bf = ml_dtypes.bfloat16
    ident = np.eye(P, dtype=np.float32)
    sel = _make_sel().astype(bf)
    in_maps = []
    for c in range(8):
        b, hh = c // 2, c % 2
        in_maps.append({
            "hs": hidden_states[b].T.astype(bf),
            "wq": np.ascontiguousarray(Wq[:, hh * JL:(hh + 1) * JL]).astype(bf),
            "wk": np.ascontiguousarray(Wk[:, hh * JL:(hh + 1) * JL]).astype(bf),
            "wv": np.ascontiguousarray(Wv[:, hh * JL:(hh + 1) * JL]).astype(bf),
            "ws": np.ascontiguousarray(Ws[:, hh * GL:(hh + 1) * GL]).astype(bf),
            "bq": np.ascontiguousarray(bq[hh * JL:(hh + 1) * JL]).astype(np.float32),
            "bk": np.ascontiguousarray(bk[hh * JL:(hh + 1) * JL]).astype(np.float32),
            "bv": np.ascontiguousarray(bv[hh * JL:(hh + 1) * JL]).astype(np.float32),
            "bs": np.ascontiguousarray(bs[hh * GL:(hh + 1) * GL]).astype(np.float32),
            "ident": ident,
            "sel": sel,
        })
    return in_maps


def kernel(hidden_states, Wq, bq, Wk, bk, Wv, bv, Ws, bs):
    B, S, H = hidden_states.shape
    NH = 16
    JL = 512
    nc = _get_nc()
    in_maps = _shard(dict(hidden_states=hidden_states, Wq=Wq, bq=bq, Wk=Wk,
                          bk=bk, Wv=Wv, bv=bv, Ws=Ws, bs=bs))

    res = bass_utils.run_bass_kernel_spmd(nc, in_maps, core_ids=list(range(8)))

    outp = np.zeros((B, S, NH * HD), dtype=np.float32)
    for c in range(8):
        b, hh = c // 2, c % 2
        outp[b][:, hh * JL:(hh + 1) * JL] = res.results[c]["out"]
    return outp



# revision 20
# speedup vs baseline: 1.3602x; 1.0520x over previous
"""Trainium2 Bass kernel for CurvSelfAttention.

Reference computation (per batch b):
    Q = hs @ Wq + bq ; K = hs @ Wk + bk ; V = hs @ Wv + bv      # [S, H]
    s = sigmoid(hs @ Ws + bs) * 0.2 + 0.9                        # [S, NH*G]
    Q[:, h*64+g*8+r] *= s[:, h*8+g]
    per head h: ctx_h = softmax(Q_h K_h^T / 8) V_h               # [S, 64]
    out = concat_h(ctx_h)                                        # [S, NH*64]

Sharding over 8 cores: core c = (b = c // 2, hh = c % 2); each core owns
batch b and heads hh*8 .. hh*8+8 (512 output columns). No collectives.

Per-core algorithm (HW-measured design rules: per-MM cost = N/2.4 + LDW,
LDW = cols/(4 if K==128 else 1)/1.2 and never hides; EXP = (N+~330)/1.2):

  hsT[k, t]       <- xbar-DMA-transpose of hs (bf16)
  s^T compact     <- Ws^T-stationary matmuls -> sigmoid -> sxc [65, S]
                     (row 64 = ones); expanded x8 + affine (0.2x+0.9) via a
                     selection-matrix matmul (sel [65,128] per jb), folded
                     into the SEL entries - no DRAM bounce.
  K^T [j, t]      <- W^T-stationary matmuls (pair rows: h0 d 0-63, h1 64-127)
  Q^T zero-padded <- same matmuls, epilogue writes (ps+bq)*sexp into
                     q0pad rows 0-63 (rows 64-127 = 0) and q1pad rows 64-127.
                     This lets the scores stationary be the full [128,128]
                     K-pair chunk (FWL fast weight load: 27ns vs 107ns).
  V [t, j]        <- hsT-chunk-stationary matmuls + ones col (denominator)
  scores psum     [128t, 1024]: head0 cols 0-511, head1 512-1023 (2 banks)
  probs           ONE Exp N=1024 per t-chunk covering both heads
  ctx^T [65, 512] per head: [V_h | 1]^T @ probs_h, accumulated over t-chunks
  out             PE-transpose chunks, * 1/denom, DMA per (hp, ssb) window

Loop: hp (4 head pairs) x ssb (4 windows of 512 s) x tc (16 t-chunks).
Projection matmuls for later head pairs drain as fillers inside the loop.
"""

import os
import sys

sys.path.insert(0, "/opt/trn_rl_repo")

import numpy as np
import ml_dtypes
from collections import deque
from contextlib import ExitStack

import concourse.bass as bass
import concourse.bacc as bacc
import concourse.tile as tile
from concourse import mybir
from concourse import bass_utils

F32 = mybir.dt.float32
BF16 = mybir.dt.bfloat16
AF = mybir.ActivationFunctionType
ALU = mybir.AluOpType

P = 128          # SBUF partitions
NB = 512         # matmul moving free-dim block
W2 = 512         # attention s-window
HD = 64          # head dim
G = 8            # groups per head
RING = 6         # probs ring chunks
LAG = 3          # ctx trails exp by this many chunks
SC_MIN, SC_MAX = 0.9, 1.1


def build_bass(S=2048, H=1024, NHL=8):
    """Build the per-core Bass module. NHL = local heads; JL = NHL*64."""
    JL = NHL * HD
    GL = NHL * G           # compact scale channels (64)
    KT = H // P            # contraction k-tiles (8)
    JB = JL // P           # j row-blocks == head pairs (4)
    NTB = S // NB          # 512-wide t blocks (4)
    NTC = S // P           # 128-wide t chunks (16)
    NSSB = S // W2         # s windows per head pair (4)
    HP = NHL // 2          # head pairs (4)

    nc = bacc.Bacc(trn_type="TRN2", target_bir_lowering=False, debug=False,
                   num_devices=8)

    hs = nc.dram_tensor("hs", [H, S], BF16, kind="ExternalInput").ap()
    wq = nc.dram_tensor("wq", [H, JL], BF16, kind="ExternalInput").ap()
    wk = nc.dram_tensor("wk", [H, JL], BF16, kind="ExternalInput").ap()
    wv = nc.dram_tensor("wv", [H, JL], BF16, kind="ExternalInput").ap()
    ws = nc.dram_tensor("ws", [H, GL], BF16, kind="ExternalInput").ap()
    bq = nc.dram_tensor("bq", [JL], F32, kind="ExternalInput").ap()
    bk = nc.dram_tensor("bk", [JL], F32, kind="ExternalInput").ap()
    bv = nc.dram_tensor("bv", [JL], F32, kind="ExternalInput").ap()
    bs = nc.dram_tensor("bs", [GL], F32, kind="ExternalInput").ap()
    sel = nc.dram_tensor("sel", [GL + 1, JL], BF16, kind="ExternalInput").ap()
    out = nc.dram_tensor("out", [S, JL], F32, kind="ExternalOutput").ap()

    with tile.TileContext(nc) as tc, ExitStack() as ctx:
        cpool = ctx.enter_context(tc.tile_pool(name="consts", bufs=1))
        qkpool = ctx.enter_context(tc.tile_pool(name="qk", bufs=1))
        vpool = ctx.enter_context(tc.tile_pool(name="v", bufs=1))
        sxpool = ctx.enter_context(tc.tile_pool(name="sexp", bufs=1))
        hpool = ctx.enter_context(tc.tile_pool(name="hsT", bufs=1))
        wpool = ctx.enter_context(tc.tile_pool(name="wts", bufs=1))
        ppsum = ctx.enter_context(tc.tile_pool(name="ppsum", bufs=2,
                                               space="PSUM"))
        ptmp = ctx.enter_context(tc.tile_pool(name="ptmp", bufs=2))

        # persistent activation tensors
        # qpad[hp]: [P, NSSB, 2, W2]; window ssb holds [q0 | q1] side by side
        # so ONE N=1024 scores matmul covers both heads. q0 slot: rows 0-63
        # live (head 2hp), rows 64-127 zero; q1 slot: rows 64-127 live.
        qpad = [qkpool.tile([P, NSSB, 2, W2], BF16, tag=f"qp{hp}",
                            name=f"qp{hp}") for hp in range(HP)]
        k_sb = [qkpool.tile([P, S], BF16, tag=f"k{jb}", name=f"k{jb}")
                for jb in range(JB)]
        # V as [t-chunk][128, NHL, 65]; col 64 of each head = ones (denom)
        v_sb = [vpool.tile([P, NHL, HD + 1], BF16, tag=f"v{i}", name=f"v{i}")
                for i in range(NTC)]
        sexp = [sxpool.tile([P, S], BF16, tag=f"sx{jb}", name=f"sx{jb}")
                for jb in range(JB)]
        sxc = ptmp.tile([GL + 1, S], BF16, tag="sxc", bufs=1)

        # ---- input loads. hs arrives pre-transposed from the host
        # ([H, S]), so there are NO DMA transposes and the three DMA-capable
        # queues (sync, scalar/ACT, gpsimd) stream in parallel:
        #   sync:   hsT chunks (4MB, the bulk)
        #   scalar: ws, bs, wk, bk, sel (scale/sigmoid + K path)
        #   gpsimd: early memsets, wq, bq, wv, bvb, idf, late memsets
        hsT = []
        for k in range(KT):
            t = hpool.tile([P, S], BF16, tag=f"hsT{k}", name=f"hsT{k}")
            nc.sync.dma_start(t[:], hs[k * P:(k + 1) * P, :])
            hsT.append(t)
        ws_sb = []
        for k in range(KT):
            t = wpool.tile([P, GL], BF16, tag=f"ws{k}", name=f"ws{k}")
            nc.scalar.dma_start(t[:], ws[k * P:(k + 1) * P, :])
            ws_sb.append(t)
        bs_sb = cpool.tile([GL, 1], F32, tag="bs")
        nc.scalar.dma_start(bs_sb[:], bs.rearrange("(a b) -> a b", b=1))
        wq_sb, wk_sb, wv_sb = [None] * KT, [None] * KT, [None] * KT
        bk_sb = []
        for k in range(KT):
            t = wpool.tile([P, JL], BF16, tag=f"wk{k}", name=f"wk{k}")
            nc.scalar.dma_start(t[:], wk[k * P:(k + 1) * P, :])
            wk_sb[k] = t
        for jb in range(JB):
            t = cpool.tile([P, 1], F32, tag=f"bk{jb}")
            nc.scalar.dma_start(
                t[:], bk[jb * P:(jb + 1) * P].rearrange("(a b) -> a b", b=1))
            bk_sb.append(t)
        sel_sb = cpool.tile([GL + 1, JL], BF16, tag="sel")
        nc.scalar.dma_start(sel_sb[:], sel)

        # gpsimd queue: memsets needed first, then Q/V-side loads, then the
        # remaining one-time memsets
        nc.gpsimd.memset(sxc[GL:GL + 1, :], 1.0)
        nc.gpsimd.memset(qpad[0][HD:P, :, 0, :], 0.0)
        nc.gpsimd.memset(qpad[0][0:HD, :, 1, :], 0.0)
        bq_sb = []
        for k in range(KT):
            t = wpool.tile([P, JL], BF16, tag=f"wq{k}", name=f"wq{k}")
            nc.gpsimd.dma_start(t[:], wq[k * P:(k + 1) * P, :])
            wq_sb[k] = t
        for jb in range(JB):
            t = cpool.tile([P, 1], F32, tag=f"bq{jb}")
            nc.gpsimd.dma_start(
                t[:], bq[jb * P:(jb + 1) * P].rearrange("(a b) -> a b", b=1))
            bq_sb.append(t)
        for k in range(KT):
            t = wpool.tile([P, JL], BF16, tag=f"wv{k}", name=f"wv{k}")
            nc.gpsimd.dma_start(t[:], wv[k * P:(k + 1) * P, :])
            wv_sb[k] = t
        bvb = cpool.tile([P, JL], F32, tag="bvb")
        nc.gpsimd.dma_start(
            bvb[:], bv.rearrange("(a b) -> a b", a=1).broadcast_to([P, JL]))
        for hp in range(1, HP):
            nc.gpsimd.memset(qpad[hp][HD:P, :, 0, :], 0.0)
            nc.gpsimd.memset(qpad[hp][0:HD, :, 1, :], 0.0)
        for i in range(NTC):
            nc.gpsimd.memset(v_sb[i][:, :, HD], 1.0)

        # ---- projection work units (4 matmuls each), emitted either in
        # the prefix or interleaved into the attention loop ----
        open_ps = {}

        def emit_qk_unit(jb, kind, tb, half):
            wlist = wq_sb if kind == "q" else wk_sb
            key = (jb, kind, tb)
            if half == 0:
                open_ps[key] = ppsum.tile([P, NB], F32, tag="pp", name="pp")
            ps = open_ps[key]
            kh = KT // 2
            for k in range(kh * half, kh * half + kh):
                nc.tensor.matmul(
                    ps[:], wlist[k][:, jb * P:(jb + 1) * P],
                    hsT[k][:, tb * NB:(tb + 1) * NB],
                    start=(k == 0), stop=(k == KT - 1))
            if half == 1:
                del open_ps[key]
                if kind == "q":
                    nc.vector.scalar_tensor_tensor(
                        qpad[jb][0:HD, tb, 0, :], ps[0:HD, :],
                        bq_sb[jb][0:HD], sexp[jb][0:HD, tb * NB:(tb + 1) * NB],
                        ALU.add, ALU.mult)
                    nc.vector.scalar_tensor_tensor(
                        qpad[jb][HD:P, tb, 1, :], ps[HD:P, :],
                        bq_sb[jb][HD:P], sexp[jb][HD:P, tb * NB:(tb + 1) * NB],
                        ALU.add, ALU.mult)
                else:
                    nc.vector.tensor_scalar_add(
                        k_sb[jb][:, tb * NB:(tb + 1) * NB], ps[:], bk_sb[jb][:])

        def emit_v_unit(tc_, half):
            key = ("v", tc_)
            if half == 0:
                open_ps[key] = ppsum.tile([P, JL], F32, tag="pp", name="pp")
            ps = open_ps[key]
            kh = KT // 2
            for k in range(kh * half, kh * half + kh):
                nc.tensor.matmul(
                    ps[:], hsT[k][:, tc_ * P:(tc_ + 1) * P], wv_sb[k][:],
                    start=(k == 0), stop=(k == KT - 1))
            if half == 1:
                del open_ps[key]
                for h in range(NHL):
                    nc.vector.tensor_add(
                        v_sb[tc_][:, h, 0:HD], ps[:, h * HD:(h + 1) * HD],
                        bvb[:, h * HD:(h + 1) * HD])

        # ---- prefix: compact scale -> sigmoid -> SEL-expand, then K jb0
        # (all tb) and Q jb0 (tb 0,1) ----
        for tb in range(NTB):
            ps = ppsum.tile([P, NB], F32, tag="pp", name="pp")
            for k in range(KT):
                nc.tensor.matmul(
                    ps[0:GL, :], ws_sb[k][:],
                    hsT[k][:, tb * NB:(tb + 1) * NB],
                    start=(k == 0), stop=(k == KT - 1))
            nc.scalar.activation(sxc[0:GL, tb * NB:(tb + 1) * NB],
                                 ps[0:GL, :], AF.Sigmoid, bias=bs_sb[:])
        # expand x8 + affine via SEL matmul: sexp[jb] = SEL_jb^T @ sxc
        for jb in range(JB):
            for tb in range(NTB):
                ps = ppsum.tile([P, NB], F32, tag="pp", name="pp")
                nc.tensor.matmul(
                    ps[:], sel_sb[:, jb * P:(jb + 1) * P],
                    sxc[:, tb * NB:(tb + 1) * NB], start=True, stop=True)
                nc.vector.tensor_copy(sexp[jb][:, tb * NB:(tb + 1) * NB],
                                      ps[:])
        for tb in range(NTB):
            for half in range(2):
                emit_qk_unit(0, "k", tb, half)
        for tb in range(2):
            for half in range(2):
                emit_qk_unit(0, "q", tb, half)

        # deferred projection units, drained as attention filler
        projq = deque()
        for tc_ in range(NTC):
            for half in range(2):
                projq.append(("v", tc_, half))
        for tb in range(2, NTB):
            for half in range(2):
                projq.append(("qk", 0, "q", tb, half))
        for jb in range(1, JB):
            for kind in ("k", "q"):
                for tb in range(NTB):
                    for half in range(2):
                        projq.append(("qk", jb, kind, tb, half))

        def drain_proj(n):
            for _ in range(n):
                if not projq:
                    return
                u = projq.popleft()
                if u[0] == "v":
                    emit_v_unit(u[1], u[2])
                else:
                    emit_qk_unit(u[1], u[2], u[3], u[4])

        # ================= attention =================
        # PSUM: scores ring 2x2 banks + ctxT 2x1 + pp 2x1 = 8 banks
        with tc.tile_pool(name="probs", bufs=1) as prpool, \
             tc.tile_pool(name="asm", bufs=2) as apool, \
             tc.tile_pool(name="spsum", bufs=2, space="PSUM") as spsum, \
             tc.tile_pool(name="cpsum", bufs=1, space="PSUM") as cpsum, \
             tc.tile_pool(name="rtmp", bufs=2) as rtpool:

            # per-(hp, ssb) filler quota (units of 4 matmuls per chunk iter)
            quota = {}
            for hp in range(HP):
                for ssb in range(NSSB):
                    quota[(hp, ssb)] = 0
            quota[(0, 0)] = 2
            quota[(0, 1)] = 1
            quota[(0, 2)] = 1
            quota[(0, 3)] = 1
            quota[(1, 0)] = 1
            quota[(1, 1)] = 1

            # deferred per-window tail work (normalize + output DMA),
            # drained one piece per iteration of the NEXT window. ctx is
            # accumulated TRANSPOSED ([s, d], probs chunk as stationary) so
            # no PE transpose / psum evacuation is needed; col 64 of each
            # head's slice is the softmax denominator.
            tailq = deque()

            def drain_tail(n):
                for _ in range(n):
                    if not tailq:
                        return
                    tailq.popleft()()

            def make_norm_tail(i, cps_i, asm):
                def run():
                    rc = rtpool.tile([P, W2 // P], F32, tag="rc")
                    nc.vector.reciprocal(rc[:], cps_i[:, :, HD])
                    for cc in range(W2 // P):
                        nc.vector.tensor_scalar_mul(
                            asm[:, cc, i, :], cps_i[:, cc, 0:HD],
                            rc[:, cc:cc + 1])
                return run

            def make_dma_tail(hp, ssb, cc, asm):
                def run():
                    srow = ssb * W2 + cc * P
                    nc.sync.dma_start(
                        out[srow:srow + P, hp * P:(hp + 1) * P], asm[:, cc])
                return run

            for hp in range(HP):
                for ssb in range(NSSB):
                    nq = quota[(hp, ssb)]
                    pts = prpool.tile([P, RING, 2 * W2], BF16, tag="pts",
                                      name="pts")
                    cps = [cpsum.tile([P, W2 // P, HD + 1], F32, tag=f"c{i}",
                                      name=f"c{i}") for i in range(2)]

                    def ctx_chunk(tcc):
                        # start=True clears the whole PSUM bank, so only the
                        # FIRST chain touching each head's bank sends it; the
                        # other cc chains' first writes overwrite anyway
                        # (has_written was cleared bank-wide).
                        for i in range(2):
                            h = hp * 2 + i
                            for cc in range(W2 // P):
                                nc.tensor.matmul(
                                    cps[i][:, cc, :],
                                    pts[:, tcc % RING,
                                        i * W2 + cc * P:i * W2 + (cc + 1) * P],
                                    v_sb[tcc][:, h, :],
                                    start=(tcc == 0 and cc == 0),
                                    stop=(tcc == NTC - 1),
                                    skip_group_check=True)

                    for tc_ in range(NTC):
                        if tc_ >= LAG:
                            ctx_chunk(tc_ - LAG)
                        drain_tail(1)
                        drain_proj(nq)
                        sc = spsum.tile([P, 2 * W2], F32, tag="sc", name="sc")
                        for i in range(2):
                            nc.tensor.matmul(
                                sc[:, i * W2:(i + 1) * W2],
                                k_sb[hp][:, tc_ * P:(tc_ + 1) * P],
                                qpad[hp][:, ssb, i, :], start=True, stop=True)
                        nc.scalar.activation(
                            pts[:, tc_ % RING, :], sc[:], AF.Exp,
                            scale=1.0 / 8.0)
                    for tcc in range(NTC - LAG, NTC):
                        ctx_chunk(tcc)

                    asm = apool.tile([P, W2 // P, 2, HD], F32, tag="asm",
                                     name="asm")
                    for i in range(2):
                        tailq.append(make_norm_tail(i, cps[i], asm))
                    for cc in range(W2 // P):
                        tailq.append(make_dma_tail(hp, ssb, cc, asm))
            while tailq:
                tailq.popleft()()

    nc.finalize()
    return nc


_CACHE = {}


def _get_nc():
    if "nc" not in _CACHE:
        _CACHE["nc"] = build_bass()
    return _CACHE["nc"]


def _make_sel():
    """SEL [65, 512]: sexp[jb][j, t] = 0.2*sig[c(jb,j), t] + 0.9 where
    c(jb, j) = (2*jb + j//64)*8 + (j%64)//8."""
    sel = np.zeros((65, 512), dtype=np.float32)
    for jb in range(4):
        for j in range(128):
            c = (2 * jb + j // 64) * 8 + (j % 64) // 8
            sel[c, jb * 128 + j] = SC_MAX - SC_MIN
    sel[64, :] = SC_MIN
    return sel


def _shard(inputs):
    """Split full inputs into 8 per-core input maps (host-side, bf16 cast)."""
    hidden_states = inputs["hidden_states"]
    Wq, bq = inputs["Wq"], inputs["bq"]
    Wk, bk = inputs["Wk"], inputs["bk"]
    Wv, bv = inputs["Wv"], inputs["bv"]
    Ws, bs = inputs["Ws"], inputs["bs"]
    JL = 512   # output cols per core
    GL = 64    # Ws cols per core
    bf = ml_dtypes.bfloat16
    sel = _make_sel().astype(bf)
    in_maps = []
    for c in range(8):
        b, hh = c // 2, c % 2
        in_maps.append({
            "hs": hidden_states[b].T.astype(bf),
            "wq": np.ascontiguousarray(Wq[:, hh * JL:(hh + 1) * JL]).astype(bf),
            "wk": np.ascontiguousarray(Wk[:, hh * JL:(hh + 1) * JL]).astype(bf),
            "wv": np.ascontiguousarray(Wv[:, hh * JL:(hh + 1) * JL]).astype(bf),
            "ws": np.ascontiguousarray(Ws[:, hh * GL:(hh + 1) * GL]).astype(bf),
            "bq": np.ascontiguousarray(bq[hh * JL:(hh + 1) * JL]).astype(np.float32),
            "bk": np.ascontiguousarray(bk[hh * JL:(hh + 1) * JL]).astype(np.float32),
            "bv": np.ascontiguousarray(bv[hh * JL:(hh + 1) * JL]).astype(np.float32),
            "bs": np.ascontiguousarray(bs[hh * GL:(hh + 1) * GL]).astype(np.float32),
            "sel": sel,
        })
    return in_maps


def kernel(hidden_states, Wq, bq, Wk, bk, Wv, bv, Ws, bs):
    B, S, H = hidden_states.shape
    NH = 16
    JL = 512
    nc = _get_nc()
    in_maps = _shard(dict(hidden_states=hidden_states, Wq=Wq, bq=bq, Wk=Wk,
                          bk=bk, Wv=Wv, bv=bv, Ws=Ws, bs=bs))

    res = bass_utils.run_bass_kernel_spmd(nc, in_maps, core_ids=list(range(8)))

    outp = np.zeros((B, S, NH * HD), dtype=np.float32)
    for c in range(8):
        b, hh = c // 2, c % 2
        outp[b][:, hh * JL:(hh + 1) * JL] = res.results[c]["out"]
    return outp



# revision 25
# speedup vs baseline: 1.4081x; 1.0352x over previous
"""Trainium2 Bass kernel for CurvSelfAttention.

Reference computation (per batch b):
    Q = hs @ Wq + bq ; K = hs @ Wk + bk ; V = hs @ Wv + bv      # [S, H]
    s = sigmoid(hs @ Ws + bs) * 0.2 + 0.9                        # [S, NH*G]
    Q[:, h*64+g*8+r] *= s[:, h*8+g]
    per head h: ctx_h = softmax(Q_h K_h^T / 8) V_h               # [S, 64]
    out = concat_h(ctx_h)                                        # [S, NH*64]

Sharding over 8 cores: core c = (b = c // 2, hh = c % 2); each core owns
batch b and heads hh*8 .. hh*8+8 (512 output columns). No collectives.

Per-core algorithm (HW-measured design rules: per-MM cost = N/2.4 + LDW,
LDW = cols/(4 if K==128 else 1)/1.2 and never hides; EXP = (N+~330)/1.2):

  hsT[k, t]       <- xbar-DMA-transpose of hs (bf16)
  s^T compact     <- Ws^T-stationary matmuls -> sigmoid -> sxc [65, S]
                     (row 64 = ones); expanded x8 + affine (0.2x+0.9) via a
                     selection-matrix matmul (sel [65,128] per jb), folded
                     into the SEL entries - no DRAM bounce.
  K^T [j, t]      <- W^T-stationary matmuls (pair rows: h0 d 0-63, h1 64-127)
  Q^T zero-padded <- same matmuls, epilogue writes (ps+bq)*sexp into
                     q0pad rows 0-63 (rows 64-127 = 0) and q1pad rows 64-127.
                     This lets the scores stationary be the full [128,128]
                     K-pair chunk (FWL fast weight load: 27ns vs 107ns).
  V [t, j]        <- hsT-chunk-stationary matmuls + ones col (denominator)
  scores psum     [128t, 1024]: head0 cols 0-511, head1 512-1023 (2 banks)
  probs           ONE Exp N=1024 per t-chunk covering both heads
  ctx^T [65, 512] per head: [V_h | 1]^T @ probs_h, accumulated over t-chunks
  out             PE-transpose chunks, * 1/denom, DMA per (hp, ssb) window

Loop: hp (4 head pairs) x ssb (4 windows of 512 s) x tc (16 t-chunks).
Projection matmuls for later head pairs drain as fillers inside the loop.
"""

import os
import sys

sys.path.insert(0, "/opt/trn_rl_repo")

import numpy as np
import ml_dtypes
from collections import deque
from contextlib import ExitStack

import concourse.bass as bass
import concourse.bacc as bacc
import concourse.tile as tile
from concourse import mybir
from concourse import bass_utils

F32 = mybir.dt.float32
BF16 = mybir.dt.bfloat16
AF = mybir.ActivationFunctionType
ALU = mybir.AluOpType

P = 128          # SBUF partitions
NB = 512         # matmul moving free-dim block
W2 = 512         # attention s-window
HD = 64          # head dim
G = 8            # groups per head
RING = 6         # probs ring chunks
LAG = 3          # ctx trails exp by this many chunks
SC_MIN, SC_MAX = 0.9, 1.1


def build_bass(S=2048, H=1024, NHL=8):
    """Build the per-core Bass module. NHL = local heads; JL = NHL*64."""
    JL = NHL * HD
    GL = NHL * G           # compact scale channels (64)
    KT = H // P            # contraction k-tiles (8)
    JB = JL // P           # j row-blocks == head pairs (4)
    NTB = S // NB          # 512-wide t blocks (4)
    NTC = S // P           # 128-wide t chunks (16)
    NSSB = S // W2         # s windows per head pair (4)
    HP = NHL // 2          # head pairs (4)

    nc = bacc.Bacc(trn_type="TRN2", target_bir_lowering=False, debug=False,
                   num_devices=8)

    hs = nc.dram_tensor("hs", [H, S], BF16, kind="ExternalInput").ap()
    wq = nc.dram_tensor("wq", [H, JL], BF16, kind="ExternalInput").ap()
    wk = nc.dram_tensor("wk", [H, JL], BF16, kind="ExternalInput").ap()
    wv = nc.dram_tensor("wv", [H, JL], BF16, kind="ExternalInput").ap()
    ws = nc.dram_tensor("ws", [H, GL], BF16, kind="ExternalInput").ap()
    bq = nc.dram_tensor("bq", [JL], F32, kind="ExternalInput").ap()
    bk = nc.dram_tensor("bk", [JL], F32, kind="ExternalInput").ap()
    bv = nc.dram_tensor("bv", [JL], F32, kind="ExternalInput").ap()
    bs = nc.dram_tensor("bs", [GL], F32, kind="ExternalInput").ap()
    sel = nc.dram_tensor("sel", [GL + 1, JL], BF16, kind="ExternalInput").ap()
    out = nc.dram_tensor("out", [S, JL], F32, kind="ExternalOutput").ap()

    with tile.TileContext(nc) as tc, ExitStack() as ctx:
        cpool = ctx.enter_context(tc.tile_pool(name="consts", bufs=1))
        qkpool = ctx.enter_context(tc.tile_pool(name="qk", bufs=1))
        vpool = ctx.enter_context(tc.tile_pool(name="v", bufs=1))
        sxpool = ctx.enter_context(tc.tile_pool(name="sexp", bufs=1))
        hpool = ctx.enter_context(tc.tile_pool(name="hsT", bufs=1))
        wpool = ctx.enter_context(tc.tile_pool(name="wts", bufs=1))
        ppsum = ctx.enter_context(tc.tile_pool(name="ppsum", bufs=2,
                                               space="PSUM"))
        ptmp = ctx.enter_context(tc.tile_pool(name="ptmp", bufs=2))

        # persistent activation tensors
        # qpad[hp]: [P, NSSB, 2, W2]; window ssb holds [q0 | q1] side by side
        # so ONE N=1024 scores matmul covers both heads. q0 slot: rows 0-63
        # live (head 2hp), rows 64-127 zero; q1 slot: rows 64-127 live.
        qpad = [qkpool.tile([P, NSSB, 2, W2], BF16, tag=f"qp{hp}",
                            name=f"qp{hp}") for hp in range(HP)]
        k_sb = [qkpool.tile([P, S], BF16, tag=f"k{jb}", name=f"k{jb}")
                for jb in range(JB)]
        # V as [t-chunk][128, NHL, 65]; col 64 of each head = ones (denom)
        v_sb = [vpool.tile([P, NHL, HD + 1], BF16, tag=f"v{i}", name=f"v{i}")
                for i in range(NTC)]
        sexp = [sxpool.tile([P, S], BF16, tag=f"sx{jb}", name=f"sx{jb}")
                for jb in range(JB)]
        sxc = ptmp.tile([GL + 1, S], BF16, tag="sxc", bufs=1)

        # ---- input loads. hs arrives pre-transposed from the host ([H, S])
        # and streams as t-QUARTERS (all 8 k-chunks of quarter q land by
        # ~4.9*(q+1) us) so the scale/K/Q chains can chase arrivals. Three
        # DMA queues run in parallel:
        #   sync:   hsT quarter 0, wv+bvb, hsT quarters 1-3
        #   scalar: small consts only (keeps ACT free for tanh early)
        #   gpsimd: wk, wq (1MB each)
        # DVE (idle early) does the one-time memsets.
        hsT = [hpool.tile([P, S], BF16, tag=f"hsT{k}", name=f"hsT{k}")
               for k in range(KT)]
        wq_sb, wk_sb, wv_sb = [None] * KT, [None] * KT, [None] * KT

        def load_quarter(tb):
            for k in range(KT):
                nc.sync.dma_start(hsT[k][:, tb * NB:(tb + 1) * NB],
                                  hs[k * P:(k + 1) * P, tb * NB:(tb + 1) * NB])

        load_quarter(0)
        for k in range(KT):
            t = wpool.tile([P, JL], BF16, tag=f"wv{k}", name=f"wv{k}")
            nc.sync.dma_start(t[:], wv[k * P:(k + 1) * P, :])
            wv_sb[k] = t
        bvb = cpool.tile([P, JL], F32, tag="bvb")
        nc.sync.dma_start(
            bvb[:], bv.rearrange("(a b) -> a b", a=1).broadcast_to([P, JL]))
        for tb in range(1, NTB):
            load_quarter(tb)

        # scalar queue: small consts (done in ~4us, then ACT loads its
        # exp/tanh table once — the ONLY table set used: sigmoid is computed
        # as 0.5*tanh(x/2)+0.5 with the affine folded into SEL)
        ws_sb = []
        for k in range(KT):
            t = wpool.tile([P, GL], BF16, tag=f"ws{k}", name=f"ws{k}")
            nc.scalar.dma_start(t[:], ws[k * P:(k + 1) * P, :])
            ws_sb.append(t)
        bs_sb = cpool.tile([GL, 1], F32, tag="bs")
        nc.scalar.dma_start(bs_sb[:], bs.rearrange("(a b) -> a b", b=1))
        sel_sb = cpool.tile([GL + 1, JL], BF16, tag="sel")
        nc.scalar.dma_start(sel_sb[:], sel)
        bk_sb = []
        bq_sb = []
        for jb in range(JB):
            t = cpool.tile([P, 1], F32, tag=f"bk{jb}")
            nc.scalar.dma_start(
                t[:], bk[jb * P:(jb + 1) * P].rearrange("(a b) -> a b", b=1))
            bk_sb.append(t)
            t = cpool.tile([P, 1], F32, tag=f"bq{jb}")
            nc.scalar.dma_start(
                t[:], bq[jb * P:(jb + 1) * P].rearrange("(a b) -> a b", b=1))
            bq_sb.append(t)

        # gpsimd queue: the K and Q weights
        for k in range(KT):
            t = wpool.tile([P, JL], BF16, tag=f"wk{k}", name=f"wk{k}")
            nc.gpsimd.dma_start(t[:], wk[k * P:(k + 1) * P, :])
            wk_sb[k] = t
        for k in range(KT):
            t = wpool.tile([P, JL], BF16, tag=f"wq{k}", name=f"wq{k}")
            nc.gpsimd.dma_start(t[:], wq[k * P:(k + 1) * P, :])
            wq_sb[k] = t

        # one-time memsets on the (idle-early) vector engine
        nc.vector.memset(sxc[GL:GL + 1, :], 1.0)
        for hp in range(HP):
            nc.vector.memset(qpad[hp][HD:P, :, 0, :], 0.0)
            nc.vector.memset(qpad[hp][0:HD, :, 1, :], 0.0)
        for i in range(NTC):
            nc.vector.memset(v_sb[i][:, :, HD], 1.0)

        # ---- projection work units (4 matmuls each), emitted either in
        # the prefix or interleaved into the attention loop ----
        open_ps = {}

        def emit_qk_unit(jb, kind, tb, half):
            wlist = wq_sb if kind == "q" else wk_sb
            key = (jb, kind, tb)
            if half == 0:
                open_ps[key] = ppsum.tile([P, NB], F32, tag="pp", name="pp")
            ps = open_ps[key]
            kh = KT // 2
            for k in range(kh * half, kh * half + kh):
                nc.tensor.matmul(
                    ps[:], wlist[k][:, jb * P:(jb + 1) * P],
                    hsT[k][:, tb * NB:(tb + 1) * NB],
                    start=(k == 0), stop=(k == KT - 1))
            if half == 1:
                del open_ps[key]
                if kind == "q":
                    nc.vector.scalar_tensor_tensor(
                        qpad[jb][0:HD, tb, 0, :], ps[0:HD, :],
                        bq_sb[jb][0:HD], sexp[jb][0:HD, tb * NB:(tb + 1) * NB],
                        ALU.add, ALU.mult)
                    nc.vector.scalar_tensor_tensor(
                        qpad[jb][HD:P, tb, 1, :], ps[HD:P, :],
                        bq_sb[jb][HD:P], sexp[jb][HD:P, tb * NB:(tb + 1) * NB],
                        ALU.add, ALU.mult)
                else:
                    nc.vector.tensor_scalar_add(
                        k_sb[jb][:, tb * NB:(tb + 1) * NB], ps[:], bk_sb[jb][:])

        def emit_v_unit(tc_, half):
            key = ("v", tc_)
            if half == 0:
                open_ps[key] = ppsum.tile([P, JL], F32, tag="pp", name="pp")
            ps = open_ps[key]
            kh = KT // 2
            for k in range(kh * half, kh * half + kh):
                nc.tensor.matmul(
                    ps[:], hsT[k][:, tc_ * P:(tc_ + 1) * P], wv_sb[k][:],
                    start=(k == 0), stop=(k == KT - 1))
            if half == 1:
                del open_ps[key]
                for h in range(NHL):
                    nc.vector.tensor_add(
                        v_sb[tc_][:, h, 0:HD], ps[:, h * HD:(h + 1) * HD],
                        bvb[:, h * HD:(h + 1) * HD])

        # ---- prefix: compact scale -> tanh (sigmoid via tanh identity; the
        # 0.5*x+0.5 affine is folded into SEL host-side; bias arrives as
        # bs/2) -> SEL-expand, then K jb0 (all tb) and Q jb0 (tb 0,1) ----
        for tb in range(NTB):
            ps = ppsum.tile([P, NB], F32, tag="pp", name="pp")
            for k in range(KT):
                nc.tensor.matmul(
                    ps[0:GL, :], ws_sb[k][:],
                    hsT[k][:, tb * NB:(tb + 1) * NB],
                    start=(k == 0), stop=(k == KT - 1))
            nc.scalar.activation(sxc[0:GL, tb * NB:(tb + 1) * NB],
                                 ps[0:GL, :], AF.Tanh, bias=bs_sb[:],
                                 scale=0.5)
        # expand x8 + affine via SEL matmul: sexp[jb] = SEL_jb^T @ sxc
        for jb in range(JB):
            for tb in range(NTB):
                ps = ppsum.tile([P, NB], F32, tag="pp", name="pp")
                nc.tensor.matmul(
                    ps[:], sel_sb[:, jb * P:(jb + 1) * P],
                    sxc[:, tb * NB:(tb + 1) * NB], start=True, stop=True)
                nc.vector.tensor_copy(sexp[jb][:, tb * NB:(tb + 1) * NB],
                                      ps[:])
        for tb in range(NTB):
            for half in range(2):
                emit_qk_unit(0, "k", tb, half)
        for tb in range(2):
            for half in range(2):
                emit_qk_unit(0, "q", tb, half)

        # deferred projection units, drained as attention filler
        projq = deque()
        for tc_ in range(NTC):
            for half in range(2):
                projq.append(("v", tc_, half))
        for tb in range(2, NTB):
            for half in range(2):
                projq.append(("qk", 0, "q", tb, half))
        for jb in range(1, JB):
            for kind in ("k", "q"):
                for tb in range(NTB):
                    for half in range(2):
                        projq.append(("qk", jb, kind, tb, half))

        def drain_proj(n):
            for _ in range(n):
                if not projq:
                    return
                u = projq.popleft()
                if u[0] == "v":
                    emit_v_unit(u[1], u[2])
                else:
                    emit_qk_unit(u[1], u[2], u[3], u[4])

        # ================= attention =================
        # PSUM: scores ring 2x2 banks + ctxT 2x1 + pp 2x1 = 8 banks
        with tc.tile_pool(name="probs", bufs=2) as prpool, \
             tc.tile_pool(name="asm", bufs=2) as apool, \
             tc.tile_pool(name="spsum", bufs=2, space="PSUM") as spsum, \
             tc.tile_pool(name="cpsum", bufs=1, space="PSUM") as cpsum, \
             tc.tile_pool(name="rtmp", bufs=2) as rtpool:

            # per-(hp, ssb) filler quota (units of 4 matmuls per chunk iter)
            quota = {}
            for hp in range(HP):
                for ssb in range(NSSB):
                    quota[(hp, ssb)] = 0
            quota[(0, 0)] = 2
            quota[(0, 1)] = 1
            quota[(0, 2)] = 1
            quota[(0, 3)] = 1
            quota[(1, 0)] = 1
            quota[(1, 1)] = 1

            # deferred per-window tail work (normalize + output DMA),
            # drained one piece per iteration of the NEXT window. ctx is
            # accumulated TRANSPOSED ([s, d], probs chunk as stationary) so
            # no PE transpose / psum evacuation is needed; col 64 of each
            # head's slice is the softmax denominator.
            tailq = deque()

            def drain_tail(n):
                for _ in range(n):
                    if not tailq:
                        return
                    tailq.popleft()()

            def make_norm_tail(i, cps_i, asm):
                def run():
                    rc = rtpool.tile([P, W2 // P], F32, tag="rc")
                    nc.vector.reciprocal(rc[:], cps_i[:, :, HD])
                    for cc in range(W2 // P):
                        nc.vector.tensor_scalar_mul(
                            asm[:, cc, i, :], cps_i[:, cc, 0:HD],
                            rc[:, cc:cc + 1])
                return run

            def make_dma_tail(hp, ssb, cc, asm):
                def run():
                    srow = ssb * W2 + cc * P
                    nc.sync.dma_start(
                        out[srow:srow + P, hp * P:(hp + 1) * P], asm[:, cc])
                return run

            for hp in range(HP):
                for ssb in range(NSSB):
                    nq = quota[(hp, ssb)]
                    pts = prpool.tile([P, RING, 2 * W2], BF16, tag="pts",
                                      name="pts")
                    cps = [cpsum.tile([P, W2 // P, HD + 1], F32, tag=f"c{i}",
                                      name=f"c{i}") for i in range(2)]

                    def ctx_chunk(tcc):
                        # start=True clears the whole PSUM bank, so only the
                        # FIRST chain touching each head's bank sends it; the
                        # other cc chains' first writes overwrite anyway
                        # (has_written was cleared bank-wide).
                        for i in range(2):
                            h = hp * 2 + i
                            for cc in range(W2 // P):
                                nc.tensor.matmul(
                                    cps[i][:, cc, :],
                                    pts[:, tcc % RING,
                                        i * W2 + cc * P:i * W2 + (cc + 1) * P],
                                    v_sb[tcc][:, h, :],
                                    start=(tcc == 0 and cc == 0),
                                    stop=(tcc == NTC - 1),
                                    skip_group_check=True)

                    for tc_ in range(NTC):
                        if tc_ >= LAG:
                            ctx_chunk(tc_ - LAG)
                        drain_tail(1)
                        drain_proj(nq)
                        sc = spsum.tile([P, 2 * W2], F32, tag="sc", name="sc")
                        for i in range(2):
                            nc.tensor.matmul(
                                sc[:, i * W2:(i + 1) * W2],
                                k_sb[hp][:, tc_ * P:(tc_ + 1) * P],
                                qpad[hp][:, ssb, i, :], start=True, stop=True)
                        nc.scalar.activation(
                            pts[:, tc_ % RING, :], sc[:], AF.Exp,
                            scale=1.0 / 8.0)
                    for tcc in range(NTC - LAG, NTC):
                        ctx_chunk(tcc)

                    asm = apool.tile([P, W2 // P, 2, HD], F32, tag="asm",
                                     name="asm")
                    for i in range(2):
                        tailq.append(make_norm_tail(i, cps[i], asm))
                    for cc in range(W2 // P):
                        tailq.append(make_dma_tail(hp, ssb, cc, asm))
            while tailq:
                tailq.popleft()()

    nc.finalize()
    return nc


_CACHE = {}


def _get_nc():
    if "nc" not in _CACHE:
        _CACHE["nc"] = build_bass()
    return _CACHE["nc"]


def _make_sel():
    """SEL [65, 512] for the tanh form: the device computes
    th = tanh((x + bs)/2) and sexp[jb][j, t] = a*th[c(jb,j), t] + m where
    a = (SC_MAX-SC_MIN)/2, m = (SC_MIN+SC_MAX)/2 (since
    sigmoid(x) = 0.5*tanh(x/2)+0.5), c(jb, j) = (2*jb + j//64)*8 + (j%64)//8.
    Row 64 multiplies the ones-row of sxc."""
    sel = np.zeros((65, 512), dtype=np.float32)
    for jb in range(4):
        for j in range(128):
            c = (2 * jb + j // 64) * 8 + (j % 64) // 8
            sel[c, jb * 128 + j] = (SC_MAX - SC_MIN) / 2
    sel[64, :] = (SC_MIN + SC_MAX) / 2
    return sel


def _shard(inputs):
    """Split full inputs into 8 per-core input maps (host-side, bf16 cast)."""
    hidden_states = inputs["hidden_states"]
    Wq, bq = inputs["Wq"], inputs["bq"]
    Wk, bk = inputs["Wk"], inputs["bk"]
    Wv, bv = inputs["Wv"], inputs["bv"]
    Ws, bs = inputs["Ws"], inputs["bs"]
    JL = 512   # output cols per core
    GL = 64    # Ws cols per core
    bf = ml_dtypes.bfloat16
    sel = _make_sel().astype(bf)
    in_maps = []
    for c in range(8):
        b, hh = c // 2, c % 2
        in_maps.append({
            "hs": hidden_states[b].T.astype(bf),
            "wq": np.ascontiguousarray(Wq[:, hh * JL:(hh + 1) * JL]).astype(bf),
            "wk": np.ascontiguousarray(Wk[:, hh * JL:(hh + 1) * JL]).astype(bf),
            "wv": np.ascontiguousarray(Wv[:, hh * JL:(hh + 1) * JL]).astype(bf),
            "ws": np.ascontiguousarray(Ws[:, hh * GL:(hh + 1) * GL]).astype(bf),
            "bq": np.ascontiguousarray(bq[hh * JL:(hh + 1) * JL]).astype(np.float32),
            "bk": np.ascontiguousarray(bk[hh * JL:(hh + 1) * JL]).astype(np.float32),
            "bv": np.ascontiguousarray(bv[hh * JL:(hh + 1) * JL]).astype(np.float32),
            "bs": (0.5 * bs[hh * GL:(hh + 1) * GL]).astype(np.float32),
            "sel": sel,
        })
    return in_maps


def kernel(hidden_states, Wq, bq, Wk, bk, Wv, bv, Ws, bs):
    B, S, H = hidden_states.shape
    NH = 16
    JL = 512
    nc = _get_nc()
    in_maps = _shard(dict(hidden_states=hidden_states, Wq=Wq, bq=bq, Wk=Wk,
                          bk=bk, Wv=Wv, bv=bv, Ws=Ws, bs=bs))

    res = bass_utils.run_bass_kernel_spmd(nc, in_maps, core_ids=list(range(8)))

    outp = np.zeros((B, S, NH * HD), dtype=np.float32)
    for c in range(8):
        b, hh = c // 2, c % 2
        outp[b][:, hh * JL:(hh + 1) * JL] = res.results[c]["out"]
    return outp

